# revision 1
# baseline (speedup 1.0000x reference)
"""ContrastiveCenterLoss on 8 Trainium2 NeuronCores.

Math: with dist[b,c] = ||f_b - c_c||^2,
  intra = sum_b dist[b, label_b]          = sum_b ||f_b - c_{label_b}||^2
  total = sum_{b,c} dist[b,c]             = C*sum||f||^2 + B*sum||c||^2 - 2*(sum_b f_b)@(sum_c c_c)
  inter = total - intra
  loss  = (1/2/B) * intra / (inter + 1e-6) / 0.1

Sharding: feat/label batch-sharded (2048 rows/core); centers statistics
sharded over 512-row slices; the full centers table stays in HBM and is
row-gathered by label via indirect DMA. Host all-reduces the per-core
partial sums in float64 and applies the final scalar division.
"""

import numpy as np

B, C, D = 16384, 4096, 128
LAMBDA_C = 1.0
NCORES = 8
BS = B // NCORES          # 2048 feat rows per core
NPT = BS // 128           # 16 feat rows per partition
NCHUNK = 4                # feat processed in 4 chunks of 512 free-dim cols
CPC = NPT // NCHUNK       # 4 row-blocks per chunk
CS = C // NCORES          # 512 center rows per core (stats slice)
CSPT = CS // 128          # 4 center rows per partition

_cached = {}


def _build_nc(repeat=1, gather_mode="indirect"):
    import concourse.bass as bass
    import concourse.tile as tile
    from concourse import bacc, mybir

    f32 = mybir.dt.float32
    i32 = mybir.dt.int32

    nc = bacc.Bacc("TRN2", target_bir_lowering=False, debug=False,
                   num_devices=NCORES)

    feat = nc.dram_tensor("feat", [BS, D], f32, kind="ExternalInput")
    labt = nc.dram_tensor("labt", [128, NPT], i32, kind="ExternalInput")
    centers = nc.dram_tensor("centers", [C, D], f32, kind="ExternalInput")
    cslice = nc.dram_tensor("cslice", [CS, D], f32, kind="ExternalInput")

    o_fsq = nc.dram_tensor("o_fsq", [128, NCHUNK], f32, kind="ExternalOutput")
    o_intra = nc.dram_tensor("o_intra", [128, NCHUNK], f32, kind="ExternalOutput")
    o_csq = nc.dram_tensor("o_csq", [128, 1], f32, kind="ExternalOutput")
    o_vec = nc.dram_tensor("o_vec", [1, 1024], f32, kind="ExternalOutput")

    CW = CPC * D  # 512 free-dim columns per chunk

    with tile.TileContext(nc) as tc:
        with tc.tile_pool(name="const", bufs=1) as cpool, \
             tc.tile_pool(name="sbuf", bufs=2) as pool, \
             tc.tile_pool(name="scratch", bufs=2) as spool, \
             tc.tile_pool(name="psum", bufs=2, space="PSUM") as psum:

            ones = cpool.tile([128, 1], f32)
            nc.vector.memset(ones[:], 1.0)

            # partition p holds feat rows p*NPT .. p*NPT+NPT-1 (contiguous 8KB)
            fv = feat.ap().rearrange("(p n) d -> p n d", p=128)
            csv = cslice.ap().rearrange("(p n) d -> p n d", p=128)

            for _ in range(repeat):
                # indices first so gathers can start early
                lab = pool.tile([128, NPT], i32, tag="lab")
                nc.sync.dma_start(out=lab[:], in_=labt.ap())

                o_fsq_t = pool.tile([128, NCHUNK], f32, tag="o_fsq_t")
                o_intra_t = pool.tile([128, NCHUNK], f32, tag="o_intra_t")
                o_csq_t = pool.tile([128, 1], f32, tag="o_csq_t")
                vec_sb = pool.tile([1, 1024], f32, tag="vec_sb")

                ps_f = psum.tile([1, CW], f32, tag="ps_f")
                ps_c = psum.tile([1, CW], f32, tag="ps_c")

                # centers-slice statistics (independent of feat path)
                cs_t = pool.tile([128, CSPT * D], f32, tag="cs_t")
                nc.sync.dma_start(out=cs_t[:], in_=csv[:, :, :])
                cs_scr = pool.tile([128, CSPT * D], f32, tag="cs_scr")
                nc.scalar.activation(out=cs_scr[:], in_=cs_t[:],
                                     func=mybir.ActivationFunctionType.Square,
                                     accum_out=o_csq_t[:, 0:1])
                nc.tensor.matmul(out=ps_c[:], lhsT=ones[:], rhs=cs_t[:],
                                 start=True, stop=True)

                for k in range(NCHUNK):
                    f_c = spool.tile([128, CW], f32, tag="f_c")
                    nc.sync.dma_start(out=f_c[:],
                                      in_=fv[:, k * CPC:(k + 1) * CPC, :])
                    cg_c = spool.tile([128, CW], f32, tag="cg_c")
                    if gather_mode == "indirect":
                        for j in range(CPC):
                            nc.gpsimd.indirect_dma_start(
                                out=cg_c[:, j * D:(j + 1) * D],
                                out_offset=None,
                                in_=centers.ap(),
                                in_offset=bass.IndirectOffsetOnAxis(
                                    ap=lab[:, k * CPC + j:k * CPC + j + 1],
                                    axis=0),
                            )
                    else:  # "fake": plain DMA of same volume (timing expt)
                        cv = centers.ap().rearrange(
                            "(q p n) d -> q p n d", p=128, n=CPC)
                        nc.sync.dma_start(out=cg_c[:], in_=cv[k])
                    # sum of f^2 on ACT
                    f_scr = spool.tile([128, CW], f32, tag="f_scr")
                    nc.scalar.activation(
                        out=f_scr[:], in_=f_c[:],
                        func=mybir.ActivationFunctionType.Square,
                        accum_out=o_fsq_t[:, k:k + 1])
                    # column sums of f on PE (accumulated over chunks)
                    nc.tensor.matmul(out=ps_f[:], lhsT=ones[:], rhs=f_c[:],
                                     start=(k == 0), stop=(k == NCHUNK - 1))
                    # intra partial on DVE: d = f - cg; accum += d*d
                    d_c = spool.tile([128, CW], f32, tag="d_c")
                    nc.vector.tensor_sub(d_c[:], f_c[:], cg_c[:])
                    d_scr = spool.tile([128, CW], f32, tag="d_scr")
                    nc.vector.scalar_tensor_tensor(
                        out=d_scr[:], in0=d_c[:], scalar=1.0, in1=d_c[:],
                        op0=mybir.AluOpType.mult, op1=mybir.AluOpType.mult,
                        accum_out=o_intra_t[:, k:k + 1])

                nc.vector.tensor_copy(vec_sb[:, 0:CW], ps_f[:])
                nc.scalar.copy(vec_sb[:, CW:2 * CW], ps_c[:])

                nc.sync.dma_start(out=o_fsq.ap(), in_=o_fsq_t[:])
                nc.sync.dma_start(out=o_intra.ap(), in_=o_intra_t[:])
                nc.sync.dma_start(out=o_csq.ap(), in_=o_csq_t[:])
                nc.sync.dma_start(out=o_vec.ap(), in_=vec_sb[:])

    nc.compile()
    return nc


def _get_nc(repeat=1, gather_mode="indirect"):
    key = ("nc", repeat, gather_mode)
    if key not in _cached:
        _cached[key] = _build_nc(repeat, gather_mode)
    return _cached[key]


def _make_in_maps(feat, label, centers):
    feat = np.ascontiguousarray(np.asarray(feat, dtype=np.float32))
    centers = np.ascontiguousarray(np.asarray(centers, dtype=np.float32))
    lab = np.asarray(label).astype(np.int32)
    in_maps = []
    for k in range(NCORES):
        fs = feat[k * BS:(k + 1) * BS]
        ls = lab[k * BS:(k + 1) * BS].reshape(128, NPT)
        cs = centers[k * CS:(k + 1) * CS]
        in_maps.append({
            "feat": np.ascontiguousarray(fs),
            "labt": np.ascontiguousarray(ls),
            "centers": centers,
            "cslice": np.ascontiguousarray(cs),
        })
    return in_maps


def _combine(results):
    sum_fsq = 0.0
    intra = 0.0
    sum_csq = 0.0
    F = np.zeros(D, dtype=np.float64)
    Cv = np.zeros(D, dtype=np.float64)
    for r in results:
        sum_fsq += r["o_fsq"].astype(np.float64).sum()
        intra += r["o_intra"].astype(np.float64).sum()
        sum_csq += r["o_csq"].astype(np.float64).sum()
        v = r["o_vec"][0].astype(np.float64)
        F += v[:512].reshape(4, 128).sum(axis=0)
        Cv += v[512:].reshape(4, 128).sum(axis=0)
    total = C * sum_fsq + B * sum_csq - 2.0 * float(F @ Cv)
    inter = total - intra
    loss = (LAMBDA_C / 2.0 / B) * intra / (inter + 1e-6) / 0.1
    return np.float32(loss)


def kernel(feat, label, centers):
    from concourse.bass_utils import run_bass_kernel_spmd

    nc = _get_nc()
    in_maps = _make_in_maps(feat, label, centers)
    res = run_bass_kernel_spmd(nc, in_maps, list(range(NCORES)))
    return _combine(res.results)



# revision 5
# speedup vs baseline: 2.0317x; 2.0317x over previous
"""ContrastiveCenterLoss on 8 Trainium2 NeuronCores.

Math: with dist[b,c] = ||f_b - c_c||^2,
  intra = sum_b dist[b, label_b]          = sum_b ||f_b - c_{label_b}||^2
  total = sum_{b,c} dist[b,c]             = C*sum||f||^2 + B*sum||c||^2 - 2*(sum_b f_b)@(sum_c c_c)
  inter = total - intra
  loss  = (1/2/B) * intra / (inter + 1e-6) / 0.1

Sharding: feat/label batch-sharded (2048 rows/core); centers statistics
sharded over 512-row slices; the full centers table stays in HBM and is
row-gathered by label via multi-index indirect DMA (bf16 cast on the way
in).  Per core the kernel emits a single [128, 16] fp32 stats tile:
per-partition partial sums of cs^2, f^2, (f-g)^2 plus the per-d column
sums F = sum_b f_b and Cs = sum_c c_c (from [128,1]-output PE matmuls).
Host all-reduces the per-core partials in float64 and applies the final
scalar division.
"""

import numpy as np

B, C, D = 16384, 4096, 128
LAMBDA_C = 1.0
NCORES = 8
BS = B // NCORES          # 2048 feat rows per core
NPT = BS // 128           # 16 feat rows per partition
CS = C // NCORES          # 512 center rows per core (stats slice)
CSPT = CS // 128          # 4 center rows per partition

# gather chunk boundaries, in slot units (16 slots of 128 rows each)
CHUNKS = (6, 12, 16)
# of each gather chunk's slots, how many get their d^2 on DVE (rest ACT)
DVE_SLOTS = (4, 4, 2)
# f^2 strategy: 'act1' one ACT op; 'act2' two; 'dve2' two DVE stt ops
F2_MODE = 'act2'

_cached = {}


def _build_nc(repeat=1, chunks=CHUNKS, dve_slots=DVE_SLOTS, f2_mode=F2_MODE):
    import concourse.bass as bass
    import concourse.tile as tile
    from concourse import bacc, mybir

    f32 = mybir.dt.float32
    bf16 = mybir.dt.bfloat16
    i32 = mybir.dt.int32
    Alu = mybir.AluOpType
    Act = mybir.ActivationFunctionType

    nc = bacc.Bacc("TRN2", target_bir_lowering=False, debug=False,
                   num_devices=NCORES)

    feat = nc.dram_tensor("feat", [BS, D], f32, kind="ExternalInput")
    labt = nc.dram_tensor("labt", [128, NPT], i32, kind="ExternalInput")
    centers = nc.dram_tensor("centers", [C, D], f32, kind="ExternalInput")
    cslice = nc.dram_tensor("cslice", [CS, D], f32, kind="ExternalInput")

    o_stats = nc.dram_tensor("o_stats", [128, 16], f32, kind="ExternalOutput")

    # stats column map
    COL_CS2 = 0
    COL_F2 = 1            # 1..2
    COL_D2 = 3            # 3 + chunk (DVE part), 8 + chunk (ACT part)
    COL_D2A = 8
    COL_F = 14
    COL_C = 15

    # partition-major views: partition p holds rows p*NPT .. p*NPT+NPT-1
    fv = feat.ap().rearrange("(p n) d -> p n d", p=128)
    csv = cslice.ap().rearrange("(p n) d -> p n d", p=128)

    bounds = [0] + list(chunks)
    nch = len(chunks)

    with tile.TileContext(nc) as tc:
        with tc.tile_pool(name="const", bufs=1) as cpool, \
             tc.tile_pool(name="sbuf", bufs=2) as pool, \
             tc.tile_pool(name="psum", bufs=2, space="PSUM") as psum:

            ones_bf = cpool.tile([128, 1], bf16)
            nc.vector.memset(ones_bf[:], 1.0)
            ones_f32 = cpool.tile([128, 1], f32)
            nc.vector.memset(ones_f32[:], 1.0)

            for _ in range(repeat):
                lab = pool.tile([128, NPT], i32, tag="lab")
                f_t = pool.tile([128, NPT * D], bf16, tag="f_t")
                cg = pool.tile([128, NPT * D], bf16, tag="cg")
                d_t = pool.tile([128, NPT * D], bf16, tag="d_t")
                cs_t = pool.tile([128, CSPT * D], f32, tag="cs_t")
                cs_scr = pool.tile([128, CSPT * D], f32, tag="cs_scr")
                f_scr = pool.tile([128, NPT * D], bf16, tag="f_scr")
                stats = pool.tile([128, 16], f32, tag="stats")

                ps_f = psum.tile([128, 1], f32, tag="ps_f")
                ps_c = psum.tile([128, 1], f32, tag="ps_c")

                # ---- loads ----
                # Pool/SWDGE: indices first (gates gathers), then feat with
                # fp32->bf16 cast on the fly.
                nc.gpsimd.dma_start(out=lab[:], in_=labt.ap())
                nc.gpsimd.dma_start(out=f_t[:], in_=fv[:, :, :])
                # HWDGE: center-slice (fp32)
                nc.sync.dma_start(out=cs_t[:], in_=csv[:, :, :])

                # Pool/SWDGE: one multi-index gather per chunk, bf16 out.
                for c in range(nch):
                    s0, s1 = bounds[c], bounds[c + 1]
                    nc.gpsimd.indirect_dma_start(
                        out=cg[:, s0 * D:s1 * D],
                        out_offset=None,
                        in_=centers.ap(),
                        in_offset=bass.IndirectOffsetOnAxis(
                            ap=lab[:, s0:s1], axis=0),
                    )

                # ---- DVE stream (in engine order) ----
                # zero unused stats columns before any accum lands
                nc.vector.memset(stats[:], 0.0)
                # cs^2 early (only needs cs_t)
                nc.vector.scalar_tensor_tensor(
                    out=cs_scr[:], in0=cs_t[:], scalar=1.0, in1=cs_t[:],
                    op0=Alu.mult, op1=Alu.mult,
                    accum_out=stats[:, COL_CS2:COL_CS2 + 1])

                if f2_mode == 'dve2':
                    for h in range(2):
                        sl = slice(h * NPT * D // 2, (h + 1) * NPT * D // 2)
                        nc.vector.scalar_tensor_tensor(
                            out=f_scr[:, sl], in0=f_t[:, sl], scalar=1.0,
                            in1=f_t[:, sl], op0=Alu.mult, op1=Alu.mult,
                            accum_out=stats[:, COL_F2 + h:COL_F2 + h + 1])

                # per-chunk: d = f - g (TT, bf16 2x), then d^2 accum split
                # DVE/ACT
                for c in range(nch):
                    s0, s1 = bounds[c], bounds[c + 1]
                    sl = slice(s0 * D, s1 * D)
                    nc.vector.tensor_tensor(
                        out=d_t[:, sl], in0=f_t[:, sl], in1=cg[:, sl],
                        op=Alu.subtract)
                    sm = s0 + dve_slots[c]
                    if sm > s0:
                        dsl = slice(s0 * D, sm * D)
                        nc.vector.scalar_tensor_tensor(
                            out=d_t[:, dsl], in0=d_t[:, dsl], scalar=1.0,
                            in1=d_t[:, dsl], op0=Alu.mult, op1=Alu.mult,
                            accum_out=stats[:, COL_D2 + c:COL_D2 + c + 1])
                    if s1 > sm:
                        asl = slice(sm * D, s1 * D)
                        nc.scalar.activation(
                            out=cg[:, asl], in_=d_t[:, asl],
                            func=Act.Square,
                            accum_out=stats[:, COL_D2A + c:COL_D2A + c + 1])

                # ---- ACT stream: f^2 ----
                if f2_mode == 'act1':
                    nc.scalar.activation(
                        out=f_scr[:], in_=f_t[:], func=Act.Square,
                        accum_out=stats[:, COL_F2:COL_F2 + 1])
                elif f2_mode == 'act2':
                    for h in range(2):
                        sl = slice(h * NPT * D // 2, (h + 1) * NPT * D // 2)
                        nc.scalar.activation(
                            out=f_scr[:, sl], in_=f_t[:, sl], func=Act.Square,
                            accum_out=stats[:, COL_F2 + h:COL_F2 + h + 1])

                # ---- PE: column sums via [128,1]-output matmuls ----
                for j in range(NPT):
                    nc.tensor.matmul(out=ps_f[:], lhsT=f_t[:, j * D:(j + 1) * D],
                                     rhs=ones_bf[:], start=(j == 0),
                                     stop=(j == NPT - 1))
                for j in range(CSPT):
                    nc.tensor.matmul(out=ps_c[:], lhsT=cs_t[:, j * D:(j + 1) * D],
                                     rhs=ones_f32[:], start=(j == 0),
                                     stop=(j == CSPT - 1))
                nc.scalar.copy(stats[:, COL_F:COL_F + 1], ps_f[:])
                nc.scalar.copy(stats[:, COL_C:COL_C + 1], ps_c[:])

                # ---- single output DMA ----
                nc.sync.dma_start(out=o_stats.ap(), in_=stats[:])

    nc.compile()
    return nc


def _get_nc(repeat=1, chunks=CHUNKS, dve_slots=DVE_SLOTS, f2_mode=F2_MODE):
    key = ("nc2", repeat, chunks, dve_slots, f2_mode)
    if key not in _cached:
        _cached[key] = _build_nc(repeat, chunks, dve_slots, f2_mode)
    return _cached[key]


def _make_in_maps(feat, label, centers):
    feat = np.ascontiguousarray(np.asarray(feat, dtype=np.float32))
    centers = np.ascontiguousarray(np.asarray(centers, dtype=np.float32))
    lab = np.asarray(label).astype(np.int32)
    in_maps = []
    for k in range(NCORES):
        fs = feat[k * BS:(k + 1) * BS]
        ls = lab[k * BS:(k + 1) * BS].reshape(128, NPT)
        cs = centers[k * CS:(k + 1) * CS]
        in_maps.append({
            "feat": np.ascontiguousarray(fs),
            "labt": np.ascontiguousarray(ls),
            "centers": centers,
            "cslice": np.ascontiguousarray(cs),
        })
    return in_maps


def _combine(results):
    nch = len(CHUNKS)
    sum_cs2 = 0.0
    sum_f2 = 0.0
    intra = 0.0
    F = np.zeros(D, dtype=np.float64)
    Cv = np.zeros(D, dtype=np.float64)
    for r in results:
        s = r["o_stats"].astype(np.float64)
        sum_cs2 += s[:, 0].sum()
        sum_f2 += s[:, 1:3].sum()
        intra += s[:, 3:3 + nch].sum() + s[:, 8:8 + nch].sum()
        F += s[:, 14]
        Cv += s[:, 15]
    total = C * sum_f2 + B * sum_cs2 - 2.0 * float(F @ Cv)
    inter = total - intra
    loss = (LAMBDA_C / 2.0 / B) * intra / (inter + 1e-6) / 0.1
    return np.float32(loss)


def kernel(feat, label, centers):
    from concourse.bass_utils import run_bass_kernel_spmd

    nc = _get_nc()
    in_maps = _make_in_maps(feat, label, centers)
    res = run_bass_kernel_spmd(nc, in_maps, list(range(NCORES)))
    return _combine(res.results)


# revision 25
# speedup vs baseline: 2.2644x; 1.1146x over previous
"""ContrastiveCenterLoss on 8 Trainium2 NeuronCores.

Math: with dist[b,c] = ||f_b - c_c||^2,
  intra = sum_b dist[b, label_b]          = sum_b ||f_b - c_{label_b}||^2
  total = sum_{b,c} dist[b,c]             = C*sum||f||^2 + B*sum||c||^2 - 2*(sum_b f_b)@(sum_c c_c)
  inter = total - intra
  loss  = (1/2/B) * intra / (inter + 1e-6) / 0.1

Sharding: feat/label batch-sharded (2048 rows/core); centers statistics
sharded over 512-row slices; the full centers table stays in HBM and is
row-gathered by label via multi-index indirect DMA (bf16 cast on the way
in).  Per core the kernel emits a single [128, 16] fp32 stats tile:
per-partition partial sums of cs^2, f^2, (f-g)^2 plus the per-d column
sums F = sum_b f_b and Cs = sum_c c_c (from [128,1]-output PE matmuls).
Host all-reduces the per-core partials in float64 and applies the final
scalar division.
"""

import numpy as np

B, C, D = 16384, 4096, 128
LAMBDA_C = 1.0
NCORES = 8
BS = B // NCORES          # 2048 feat rows per core
NPT = BS // 128           # 16 feat rows per partition
CS = C // NCORES          # 512 center rows per core (stats slice)
CSPT = CS // 128          # 4 center rows per partition

# tuned via TimelineSim search (see test.py / search harness)
CFG = dict(
    chunks=(10, 16),      # gather chunk boundaries in slot units
    # per chunk: list of (sub_engine, d2_engine, nslots) pieces.
    # sub_engine: 'dve' (TT bf16 2x) or 'pool' (gpsimd TT).
    # d2_engine: 'dve' (stt accum) or 'act' (Square accum).
    # tuned via TimelineSim search (search.py / search3.py)
    pieces=(
        (('dve', 'act', 5), ('dve', 'act', 5)),
        (('dve', 'dve', 6),),
    ),
    f2_dve_slots=4,       # slots of f^2 on DVE (rest ACT)
    f2_act_slots=12,
    labt_engine='sync',   # 'sync' (HWDGE) or 'gpsimd' (SWDGE)
    cs2_engine='dve',     # 'dve' | 'act'
    f_t_split=8,          # slots in first feat DMA (0 = single DMA)
    idx_from='sbuf',      # 'sbuf' (DMA labels in first) or 'dram' (direct)
)

_cached = {}


def _build_nc(repeat=1, cfg=None):
    import concourse.bass as bass
    import concourse.tile as tile
    from concourse import bacc, mybir

    if cfg is None:
        cfg = CFG
    chunks = cfg['chunks']
    pieces = cfg['pieces']
    sizes = [b - a for a, b in zip((0,) + tuple(chunks[:-1]), chunks)]
    f2_dve_slots = cfg['f2_dve_slots']
    f2_act_slots = cfg.get('f2_act_slots', NPT - cfg['f2_dve_slots'])
    labt_engine = cfg['labt_engine']
    cs2_engine = cfg['cs2_engine']
    idx_from = cfg.get('idx_from', 'sbuf')
    f_t_split = cfg.get('f_t_split', 0)   # slots in first feat DMA (0=one)
    assert len(pieces) == len(chunks)
    for s, pl in zip(sizes, pieces):
        assert sum(p[2] for p in pl) == s, (sizes, pieces)
        # d^2 accum must be DVE or ACT (gpsimd accum fails neuronxcc)
        assert all(p[1] in ('dve', 'act') for p in pl)
        assert all(p[0] in ('dve', 'pool') for p in pl)
    n_pieces = sum(len(pl) for pl in pieces)
    assert n_pieces <= 9, "not enough stats columns (3..11 for d^2)"
    assert f2_dve_slots + f2_act_slots <= NPT

    f32 = mybir.dt.float32
    bf16 = mybir.dt.bfloat16
    i32 = mybir.dt.int32
    Alu = mybir.AluOpType
    Act = mybir.ActivationFunctionType

    nc = bacc.Bacc("TRN2", target_bir_lowering=False, debug=False,
                   num_devices=NCORES)

    feat = nc.dram_tensor("feat", [BS, D], f32, kind="ExternalInput")
    labt = nc.dram_tensor("labt", [128, NPT], i32, kind="ExternalInput")
    centers = nc.dram_tensor("centers", [C, D], f32, kind="ExternalInput")
    cslice = nc.dram_tensor("cslice", [CS, D], f32, kind="ExternalInput")

    o_stats = nc.dram_tensor("o_stats", [128, 16], f32, kind="ExternalOutput")

    # stats column map; host sums cols 3..13 as intra, 1..2 (+13) as f^2
    COL_CS2 = 0
    COL_F2 = 1            # 1 dve, 2 act
    COL_D2 = 3            # 3 + chunk (DVE), 6 + chunk (ACT), 9 + chunk (Pool)
    COL_D2A = 6
    COL_D2P = 9
    COL_F2P = 12          # f^2 pool part
    COL_F = 14
    COL_C = 15

    # partition-major views: partition p holds rows p*NPT .. p*NPT+NPT-1
    fv = feat.ap().rearrange("(p n) d -> p n d", p=128)
    csv = cslice.ap().rearrange("(p n) d -> p n d", p=128)

    bounds = [0] + list(chunks)
    nch = len(chunks)

    with tile.TileContext(nc) as tc:
        with tc.tile_pool(name="const", bufs=1) as cpool, \
             tc.tile_pool(name="sbuf", bufs=2) as pool, \
             tc.tile_pool(name="psum", bufs=2, space="PSUM") as psum:

            ones_bf = cpool.tile([128, 1], bf16)
            nc.vector.memset(ones_bf[:], 1.0)
            ones_f32 = cpool.tile([128, 1], f32)
            nc.vector.memset(ones_f32[:], 1.0)
            # warm the ACT Square table during the load phase: the table
            # load costs ~1.3us and otherwise lands right before the first
            # real Square, stalling the ACT stream mid-kernel.
            warm = cpool.tile([128, 1], f32)
            nc.scalar.activation(
                out=warm[:], in_=ones_f32[:],
                func=mybir.ActivationFunctionType.Square)

            for _ in range(repeat):
                lab = pool.tile([128, NPT], i32, tag="lab")
                f_t = pool.tile([128, NPT * D], bf16, tag="f_t")
                cg = pool.tile([128, NPT * D], bf16, tag="cg")
                d_t = pool.tile([128, NPT * D], bf16, tag="d_t")
                cs_t = pool.tile([128, CSPT * D], f32, tag="cs_t")
                cs_scr = pool.tile([128, CSPT * D], f32, tag="cs_scr")
                f_scr = pool.tile([128, NPT * D], bf16, tag="f_scr")
                stats = pool.tile([128, 16], f32, tag="stats")

                ps_f = psum.tile([128, 1], f32, tag="ps_f")
                ps_c = psum.tile([128, 1], f32, tag="ps_c")

                # ---- loads ----
                if idx_from == 'sbuf':
                    if labt_engine == 'sync':
                        nc.sync.dma_start(out=lab[:], in_=labt.ap())
                    else:
                        nc.gpsimd.dma_start(out=lab[:], in_=labt.ap())
                # Pool/SWDGE: feat with fp32->bf16 cast on the fly,
                # optionally in two pieces so compute can start earlier
                # (and the cs transfer can slot between them).
                if f_t_split > 0:
                    nc.gpsimd.dma_start(out=f_t[:, :f_t_split * D],
                                        in_=fv[:, :f_t_split, :])
                    nc.gpsimd.dma_start(out=f_t[:, f_t_split * D:],
                                        in_=fv[:, f_t_split:, :])
                else:
                    nc.gpsimd.dma_start(out=f_t[:], in_=fv[:, :, :])
                # HWDGE: center-slice (fp32)
                nc.sync.dma_start(out=cs_t[:], in_=csv[:, :, :])

                # Pool/SWDGE: one multi-index gather per chunk, bf16 out.
                # Index array read straight from DRAM in 'dram' mode: the
                # ExternalInput is staged in HBM before kernel start, so the
                # gather emission has no DMA dependency to wait on.
                for c in range(nch):
                    s0, s1 = bounds[c], bounds[c + 1]
                    idx_ap = (labt.ap()[:, s0:s1] if idx_from == 'dram'
                              else lab[:, s0:s1])
                    nc.gpsimd.indirect_dma_start(
                        out=cg[:, s0 * D:s1 * D],
                        out_offset=None,
                        in_=centers.ap(),
                        in_offset=bass.IndirectOffsetOnAxis(
                            ap=idx_ap, axis=0),
                    )

                # ---- elementwise reductions (3-way DVE/ACT/Pool split) ----
                # zero unused stats columns before any accum lands
                nc.vector.memset(stats[:], 0.0)

                # f^2: DVE part first (feat lands before cs in the DMA
                # queue), then ACT part split at the f_t_split boundary so
                # ACT can start on the first feat half early.
                if f2_dve_slots > 0:
                    sl = slice(0, f2_dve_slots * D)
                    nc.vector.scalar_tensor_tensor(
                        out=f_scr[:, sl], in0=f_t[:, sl], scalar=1.0,
                        in1=f_t[:, sl], op0=Alu.mult, op1=Alu.mult,
                        accum_out=stats[:, COL_F2:COL_F2 + 1])
                a0, a1 = f2_dve_slots, f2_dve_slots + f2_act_slots
                amid = min(max(f_t_split, a0), a1) if f_t_split > 0 else a1
                if amid > a0:
                    sl = slice(a0 * D, amid * D)
                    nc.scalar.activation(
                        out=f_scr[:, sl], in_=f_t[:, sl], func=Act.Square,
                        accum_out=stats[:, COL_F2 + 1:COL_F2 + 2])
                if a1 > amid:
                    sl = slice(amid * D, a1 * D)
                    nc.scalar.activation(
                        out=f_scr[:, sl], in_=f_t[:, sl], func=Act.Square,
                        accum_out=stats[:, COL_F2P:COL_F2P + 1])

                if cs2_engine == 'dve':
                    nc.vector.scalar_tensor_tensor(
                        out=cs_scr[:], in0=cs_t[:], scalar=1.0, in1=cs_t[:],
                        op0=Alu.mult, op1=Alu.mult,
                        accum_out=stats[:, COL_CS2:COL_CS2 + 1])
                else:
                    nc.scalar.activation(
                        out=cs_scr[:], in_=cs_t[:], func=Act.Square,
                        accum_out=stats[:, COL_CS2:COL_CS2 + 1])

                # per-chunk, per-piece: d = f - g (TT), then d^2 accum.
                # Each piece gets its own stats column so the dependency
                # graph stays piece-local.
                col = COL_D2
                for c in range(nch):
                    s0 = bounds[c]
                    # subs first (in piece order), then the d^2 ops, so an
                    # engine's d^2 for piece i never blocks a later sub.
                    ranges = []
                    p0 = s0
                    for sub_e, d2_e, ns in pieces[c]:
                        sl = slice(p0 * D, (p0 + ns) * D)
                        ranges.append((sl, d2_e))
                        eng = nc.vector if sub_e == 'dve' else nc.gpsimd
                        eng.tensor_tensor(
                            out=d_t[:, sl], in0=f_t[:, sl], in1=cg[:, sl],
                            op=Alu.subtract)
                        p0 += ns
                    for sl, d2_e in ranges:
                        if d2_e == 'dve':
                            nc.vector.scalar_tensor_tensor(
                                out=d_t[:, sl], in0=d_t[:, sl], scalar=1.0,
                                in1=d_t[:, sl], op0=Alu.mult, op1=Alu.mult,
                                accum_out=stats[:, col:col + 1])
                        else:
                            nc.scalar.activation(
                                out=cg[:, sl], in_=d_t[:, sl],
                                func=Act.Square,
                                accum_out=stats[:, col:col + 1])
                        col += 1

                # ---- PE: column sums via [128,1]-output matmuls ----
                for j in range(CSPT):
                    nc.tensor.matmul(out=ps_c[:], lhsT=cs_t[:, j * D:(j + 1) * D],
                                     rhs=ones_f32[:], start=(j == 0),
                                     stop=(j == CSPT - 1))
                for j in range(NPT):
                    nc.tensor.matmul(out=ps_f[:], lhsT=f_t[:, j * D:(j + 1) * D],
                                     rhs=ones_bf[:], start=(j == 0),
                                     stop=(j == NPT - 1))
                nc.scalar.copy(stats[:, COL_C:COL_C + 1], ps_c[:])
                nc.scalar.copy(stats[:, COL_F:COL_F + 1], ps_f[:])

                # ---- single output DMA ----
                nc.sync.dma_start(out=o_stats.ap(), in_=stats[:])

    nc.compile()
    return nc


def _get_nc(repeat=1, cfg=None):
    if cfg is None:
        cfg = CFG
    key = ("nc3", repeat, tuple(sorted(
        (k, tuple(v) if isinstance(v, (list, tuple)) else v)
        for k, v in cfg.items())))
    if key not in _cached:
        _cached[key] = _build_nc(repeat, cfg)
    return _cached[key]


def _make_in_maps(feat, label, centers):
    feat = np.ascontiguousarray(np.asarray(feat, dtype=np.float32))
    centers = np.ascontiguousarray(np.asarray(centers, dtype=np.float32))
    lab = np.asarray(label).astype(np.int32)
    in_maps = []
    for k in range(NCORES):
        fs = feat[k * BS:(k + 1) * BS]
        ls = lab[k * BS:(k + 1) * BS].reshape(128, NPT)
        cs = centers[k * CS:(k + 1) * CS]
        in_maps.append({
            "feat": np.ascontiguousarray(fs),
            "labt": np.ascontiguousarray(ls),
            "centers": centers,
            "cslice": np.ascontiguousarray(cs),
        })
    return in_maps


def _combine(results):
    sum_cs2 = 0.0
    sum_f2 = 0.0
    intra = 0.0
    F = np.zeros(D, dtype=np.float64)
    Cv = np.zeros(D, dtype=np.float64)
    for r in results:
        s = r["o_stats"].astype(np.float64)
        sum_cs2 += s[:, 0].sum()
        sum_f2 += s[:, 1:3].sum() + s[:, 12].sum()
        intra += s[:, 3:12].sum()
        F += s[:, 14]
        Cv += s[:, 15]
    total = C * sum_f2 + B * sum_cs2 - 2.0 * float(F @ Cv)
    inter = total - intra
    loss = (LAMBDA_C / 2.0 / B) * intra / (inter + 1e-6) / 0.1
    return np.float32(loss)


def kernel(feat, label, centers):
    from concourse.bass_utils import run_bass_kernel_spmd

    nc = _get_nc()
    in_maps = _make_in_maps(feat, label, centers)
    res = run_bass_kernel_spmd(nc, in_maps, list(range(NCORES)))
    return _combine(res.results)
